# revision 58
# baseline (speedup 1.0000x reference)
"""Luong concat attention kernel for Trainium2, data-parallel over batch on 8 cores.

Reference computation (per batch row b):
    proj   = enc[b] @ W_e + dec[b] @ W_d        # [S, A]
    energy = tanh(proj)                          # [S, A]
    scores = energy @ v                          # [S]        (output 1)
    probs  = softmax(scores)
    ctx    = probs @ enc[b]                      # [D]        (output 2)

Device dataflow (per core, 4 batches of S=4096, D=A=512):
  - proj computed transposed (projT[a, s]) on PE: lhsT = W_e chunk [d,a]
    (stationary), rhs = encT chunk [d, s] (bf16, pre-transposed on host).
    encT is the ONLY large input (17 MB/core HBM traffic).
  - dec[b] @ W_d is 0.025% of the FLOPs -> computed on host, folded into the
    tanh as the ACT engine's per-partition bias (projT has a on partitions).
  - scores = v . energyT : matvecs on PE with the stationary v broadcast to
    M in {32,64} columns so each s-tile's scores land replicated at legal
    partition offsets {0,32,64} across 3 PSUM banks (fully initialized).
  - exp on ACT with fused accum_out row sums (softmax Z partials, host-summed).
  - exp rows are replicated across partitions via K=1 PE matmuls
    (ones-row.T @ exps-row), then context = sum_s encT[d,s]*exps[s] runs as
    elementwise mult + pairwise-tree adds + one reduce, split across DVE
    (d-chunks 0-1) and GPSIMD (d-chunks 2-3).
  - Host epilogue: ctx = ctxT / Z (unshard step).
"""

import numpy as np
import ml_dtypes

import concourse.bass as bass
import concourse.bacc as bacc
import concourse.tile as tile
import concourse.mybir as mybir

BF16 = mybir.dt.bfloat16
F32 = mybir.dt.float32
AF = mybir.ActivationFunctionType

N_CORES = 8
B, S, D, A = 32, 4096, 512, 512
BPC = B // N_CORES          # batches per core = 4
P = 128
NDC = D // P                # 4 contraction chunks
NAC = A // P                # 4 a chunks
NST = S // 512              # 8 s tiles of 512

# s-tile -> (psum bank, partition row, replication M). Banks round-robin and
# adjacent s-tiles sit in different banks AND different PE column groups, so
# their score-matmul accumulation chains can run concurrently in the array.
ST_MAP = [(0, 0, 32), (1, 32, 32), (2, 64, 64), (0, 32, 32),
          (1, 64, 64), (2, 0, 64), (0, 64, 64), (1, 0, 32)]
# per bank: s-tiles sorted by partition row (probsT3 extraction order)
BANK_STS = []
for _g in range(3):
    _sts = sorted([st for st in range(NST) if ST_MAP[st][0] == _g],
                  key=lambda st: ST_MAP[st][1])
    BANK_STS.append(_sts)
# chunk col in probsT3 for s-tile st: (g*3 + idx)*4 + jj
COLBASE = {}
for _g, _sts in enumerate(BANK_STS):
    for _i, _st in enumerate(_sts):
        COLBASE[_st] = (_g * 3 + _i) * 4


def build_nc():
    nc = bacc.Bacc("TRN2", target_bir_lowering=False, debug=False,
                   num_devices=N_CORES)

    # inputs (per core)
    encT_d = nc.dram_tensor("encT", [BPC, D, S], BF16, kind="ExternalInput")
    we_d = nc.dram_tensor("we", [D, A], BF16, kind="ExternalInput")
    v_d = nc.dram_tensor("vv", [P, NAC], BF16, kind="ExternalInput")
    bias_d = nc.dram_tensor("biasT", [P, NAC * BPC], F32, kind="ExternalInput")
    ones_d = nc.dram_tensor("ones1", [P, P], BF16, kind="ExternalInput")
    id_d = nc.dram_tensor("ident", [P, P], BF16, kind="ExternalInput")
    encN3_d = nc.dram_tensor("encN3", [S, D], BF16, kind="ExternalInput")

    # outputs (per core)
    scores_o = nc.dram_tensor("scores_o", [BPC, S], F32, kind="ExternalOutput")
    ctxT_o = nc.dram_tensor("ctxT_o", [BPC - 1, P, NDC], F32,
                            kind="ExternalOutput")
    ctx3_o = nc.dram_tensor("ctx3_o", [4, D], F32, kind="ExternalOutput")
    zs_o = nc.dram_tensor("zs_o", [BPC, P, 3], F32, kind="ExternalOutput")

    with tile.TileContext(nc) as tc:
        with (
            tc.tile_pool(name="const", bufs=1) as cpool,
            tc.tile_pool(name="encT", bufs=2 * NDC) as encT_pool,
            tc.tile_pool(name="energy", bufs=8) as en_pool,
            tc.tile_pool(name="small", bufs=2) as sm_pool,
            tc.tile_pool(name="scr", bufs=12) as scr_pool,
            tc.tile_pool(name="prep", bufs=4) as prep_pool,
            tc.tile_pool(name="pproj", bufs=2, space="PSUM") as pp_pool,
            tc.tile_pool(name="pscore", bufs=3, space="PSUM") as ps_pool,
            tc.tile_pool(name="prep_ps", bufs=1, space="PSUM") as pr_pool,
        ):
            # constants
            we_sb = cpool.tile([P, NDC * A], BF16, tag="we")
            nc.sync.dma_start(
                out=we_sb[:].rearrange("p (dc a) -> p dc a", dc=NDC),
                in_=we_d[:].rearrange("(dc p) a -> p dc a", p=P),
            )
            v_sb = cpool.tile([P, NAC], BF16, tag="v")
            nc.sync.dma_start(out=v_sb[:], in_=v_d[:])
            bias_sb = cpool.tile([P, NAC * BPC], F32, tag="bias")
            nc.sync.dma_start(out=bias_sb[:], in_=bias_d[:])
            ones_sb = cpool.tile([P, P], BF16, tag="ones1")
            nc.sync.dma_start(out=ones_sb[:], in_=ones_d[:])
            id_sb = cpool.tile([P, P], BF16, tag="ident")
            nc.sync.dma_start(out=id_sb[:], in_=id_d[:])

            for b in range(BPC):
                # ---- load this batch's encoder states (fine-grained so the
                # first proj matmuls can start almost immediately) ----
                encT_sb = [encT_pool.tile([P, S], BF16, tag="encT",
                                          name=f"encT_{b}_{dc}")
                           for dc in range(NDC)]
                nseg = 8 if b == 0 else 2
                seg = S // nseg
                for h in range(nseg):
                    for dc in range(NDC):
                        nc.sync.dma_start(
                            out=encT_sb[dc][:, h * seg:(h + 1) * seg],
                            in_=encT_d[b, dc * P:(dc + 1) * P,
                                       h * seg:(h + 1) * seg])
                if b == BPC - 1:
                    # natural-layout copy of the last batch for its PE context
                    encN3_sb = []
                    for g2 in range(NST):
                        t = encT_pool.tile([P, 4 * 512], BF16, tag="encN3",
                                           name=f"encN3_{g2}")
                        nc.sync.dma_start(
                            out=t[:].rearrange("p (j d) -> p j d", j=4),
                            in_=encN3_d[g2 * 512:(g2 + 1) * 512, :].rearrange(
                                "(j p) d -> p j d", p=P),
                        )
                        encN3_sb.append(t)

                # ---- projT -> tanh -> scores ----
                sc_ps = [ps_pool.tile([P, 512], F32, tag="scores",
                                      name=f"sc_ps_{b}_{g}")
                         for g in range(3)]
                for t2 in range(NST // 2):
                    ets = []
                    for ac in range(NAC):
                        pp = pp_pool.tile([P, 1024], F32, tag="proj",
                                          name=f"pp_{b}_{t2}_{ac}")
                        for half in range(2):
                            st = 2 * t2 + half
                            for dc in range(NDC):
                                nc.tensor.matmul(
                                    pp[:, half * 512:(half + 1) * 512],
                                    lhsT=we_sb[:, dc * A + ac * P:
                                               dc * A + (ac + 1) * P],
                                    rhs=encT_sb[dc][:, st * 512:(st + 1) * 512],
                                    start=(dc == 0), stop=(dc == NDC - 1),
                                )
                        et = en_pool.tile([P, 1024], BF16, tag="energy",
                                          name=f"et_{b}_{t2}_{ac}")
                        nc.scalar.activation(
                            et[:], pp[:], AF.Tanh,
                            bias=bias_sb[:, ac * BPC + b:ac * BPC + b + 1],
                        )
                        ets.append(et)
                    # one contiguous accumulation group per psum bank
                    for half in range(2):
                        st = 2 * t2 + half
                        g, row, m = ST_MAP[st]
                        for ac in range(NAC):
                            nc.tensor.matmul(
                                sc_ps[g][row:row + m, :],
                                lhsT=v_sb[:, ac:ac + 1].to_broadcast((P, m)),
                                rhs=ets[ac][:, half * 512:(half + 1) * 512],
                                start=(ac == 0), stop=(ac == NAC - 1),
                                tile_position=(0, row),
                            )

                # ---- scores out + exp + softmax partials ----
                exps_sb = sm_pool.tile([P, 3 * 512], BF16, tag="exps")
                zacc_sb = sm_pool.tile([P, 3], F32, tag="zacc")
                for g in range(3):
                    sc_sb = sm_pool.tile([P, 512], F32, tag="scores_sb",
                                         name=f"sc_sb_{b}_{g}")
                    nc.scalar.copy(sc_sb[:], sc_ps[g][:])
                    for st in BANK_STS[g]:
                        row = ST_MAP[st][1]
                        nc.gpsimd.dma_start(
                            out=scores_o[b, st * 512:(st + 1) * 512],
                            in_=sc_sb[row:row + 1, :],
                        )
                    nc.scalar.activation(
                        exps_sb[:, g * 512:(g + 1) * 512],
                        sc_ps[g][:],
                        AF.Exp,
                        accum_out=zacc_sb[:, g:g + 1],
                    )
                nc.gpsimd.dma_start(out=zs_o[b], in_=zacc_sb[:])

                if b == BPC - 1:
                    # Last batch: its epilogue is the exposed kernel tail, so
                    # run the context matvec on the (now idle) PE instead.
                    # probsT3[:, st*4+jj] = exps for s-chunk st*4+jj, gathered
                    # by strided SBUF->SBUF DMAs from the exp rows.
                    probsT3 = sm_pool.tile([P, 32], BF16, tag="probsT3")
                    for g in range(3):
                        sts = BANK_STS[g]
                        rstride = 32 if len(sts) == 3 else 64
                        for jj in range(4):
                            pt_ps = pr_pool.tile([P, P], BF16, tag="prep_ps",
                                                 name=f"pt3_{g}_{jj}")
                            nc.tensor.transpose(
                                pt_ps[:],
                                exps_sb[:, g * 512 + jj * P:
                                        g * 512 + (jj + 1) * P],
                                id_sb[:],
                            )
                            # cols {row(st)} of the transpose are the probs
                            # for s-chunks st*4 + jj
                            nc.vector.tensor_copy(
                                probsT3[:].rearrange("p (cc jj) -> p cc jj",
                                                     jj=4)
                                [:, g * 3:g * 3 + len(sts), jj:jj + 1],
                                pt_ps[:].rearrange("p (q r) -> p q r",
                                                   r=rstride)
                                [:, 0:len(sts), 0:1],
                            )
                    ctx_ps = pr_pool.tile([P, 512], F32, tag="prep_ps",
                                          name="ctx3_ps")
                    for j in range(4):
                        for i in range(8):
                            c = j * 8 + i
                            col = COLBASE[c // 4] + c % 4
                            nc.tensor.matmul(
                                ctx_ps[32 * j:32 * j + 32, :],
                                lhsT=probsT3[:, col:col + 1].to_broadcast((P, 32)),
                                rhs=encN3_sb[c // 4][:, (c % 4) * 512:
                                                    (c % 4 + 1) * 512],
                                start=(i == 0), stop=(i == 7),
                                tile_position=(0, 32 * j),
                            )
                    ctx3_sb = sm_pool.tile([P, 512], F32, tag="ctx3_sb")
                    nc.scalar.copy(ctx3_sb[:], ctx_ps[:])
                    nc.gpsimd.dma_start(
                        out=ctx3_o[:],
                        in_=ctx3_sb[:].rearrange("(j r) f -> j (r f)",
                                                 r=32)[:, 0:512],
                    )
                    continue

                # ---- replicate exp rows across partitions (K=1 PE matmuls),
                # then context as mult + tree-add + reduce on DVE/GPSIMD ----
                prs = []
                for t2 in range(NST // 2):
                    pr2 = prep_pool.tile([P, 1024], BF16, tag="prep",
                                         name=f"pr2_{b}_{t2}")
                    for half in range(2):
                        st = 2 * t2 + half
                        g, row, _ = ST_MAP[st]
                        pr_ps = pr_pool.tile([P, 512], F32, tag="prep_ps",
                                             name=f"prps_{b}_{st}")
                        nc.tensor.matmul(
                            pr_ps[:],
                            lhsT=ones_sb[row:row + 1, :],
                            rhs=exps_sb[row:row + 1, g * 512:(g + 1) * 512],
                            start=True, stop=True,
                            tile_position=(row, 0),
                        )
                        nc.vector.tensor_copy(
                            pr2[:, half * 512:(half + 1) * 512], pr_ps[:])
                    prs.append(pr2)
                ctxT_sb = sm_pool.tile([P, NDC], F32, tag="ctxT_sb")
                for dc in range(NDC):
                    eng = nc.vector if dc < 3 else nc.gpsimd
                    scrs = []
                    for t2 in range(NST // 2):
                        scr = scr_pool.tile([P, 1024], BF16, tag="scr",
                                            name=f"scr_{b}_{t2}_{dc}")
                        eng.tensor_tensor(
                            out=scr[:],
                            in0=encT_sb[dc][:, t2 * 1024:(t2 + 1) * 1024],
                            in1=prs[t2][:],
                            op=mybir.AluOpType.mult,
                        )
                        scrs.append(scr)
                    eng.tensor_tensor(out=scrs[0][:], in0=scrs[0][:],
                                      in1=scrs[1][:], op=mybir.AluOpType.add)
                    eng.tensor_tensor(out=scrs[2][:], in0=scrs[2][:],
                                      in1=scrs[3][:], op=mybir.AluOpType.add)
                    eng.tensor_tensor(out=scrs[0][:], in0=scrs[0][:],
                                      in1=scrs[2][:], op=mybir.AluOpType.add)
                    nc.vector.reduce_sum(
                        out=ctxT_sb[:, dc:dc + 1], in_=scrs[0][:],
                        axis=mybir.AxisListType.X,
                    )
                nc.gpsimd.dma_start(out=ctxT_o[b], in_=ctxT_sb[:])

    nc.compile()
    return nc


_NC_CACHE = None


def _get_nc():
    global _NC_CACHE
    if _NC_CACHE is None:
        _NC_CACHE = build_nc()
    return _NC_CACHE


def make_in_maps(encode_state, decode_state, W, v):
    """Host-side shard + layout prep. Returns per-core input dicts."""
    bf16 = ml_dtypes.bfloat16
    W_e, W_d = W[:D], W[D:]
    dec_proj = decode_state.astype(np.float32) @ W_d.astype(np.float32)  # [B, A]
    enc_bf = encode_state.astype(bf16)
    we_bf = np.ascontiguousarray(W_e.astype(bf16))
    v_bf = np.ascontiguousarray(v.astype(bf16).reshape(NAC, P).T)        # [P, NAC]
    ones1 = np.ones((P, P), dtype=bf16)

    in_maps = []
    for c in range(N_CORES):
        sl = slice(c * BPC, (c + 1) * BPC)
        encT = np.ascontiguousarray(enc_bf[sl].transpose(0, 2, 1))       # [BPC,D,S]
        # biasT[p, ac*BPC + b] = dec_proj[core b, ac*128 + p]
        biasT = np.ascontiguousarray(
            dec_proj[sl].reshape(BPC, NAC, P).transpose(2, 1, 0).reshape(P, NAC * BPC)
        ).astype(np.float32)
        in_maps.append({
            "encT": encT, "we": we_bf, "vv": v_bf, "biasT": biasT,
            "ones1": ones1, "ident": np.eye(P, dtype=bf16),
            "encN3": np.ascontiguousarray(enc_bf[c * BPC + BPC - 1]),
        })
    return in_maps


def assemble(results):
    """Gather per-core outputs into full (scores, context_vector)."""
    scores = np.concatenate([r["scores_o"] for r in results], axis=0)
    ctx = []
    for r in results:
        zp = r["zs_o"]                                                   # [BPC,P,3]
        z = np.zeros(BPC, dtype=np.float64)
        for st in range(NST):
            g, row, _ = ST_MAP[st]
            z += zp[:, row, g]
        c = np.empty((BPC, D), dtype=np.float64)
        c[:BPC - 1] = r["ctxT_o"].transpose(0, 2, 1).reshape(BPC - 1, D)
        c[BPC - 1] = r["ctx3_o"].sum(axis=0)
        c /= z[:, None]
        ctx.append(c)
    context = np.concatenate(ctx, axis=0).astype(np.float32)
    return scores.astype(np.float32), context


def kernel(encode_state, decode_state, W, v):
    from concourse.bass_utils import run_bass_kernel_spmd
    nc = _get_nc()
    in_maps = make_in_maps(encode_state, decode_state, W, v)
    res = run_bass_kernel_spmd(nc, in_maps, list(range(N_CORES)))
    return assemble(res.results)


# revision 60
# speedup vs baseline: 1.1588x; 1.1588x over previous
"""Luong concat attention kernel for Trainium2, data-parallel over batch on 8 cores.

Reference computation (per batch row b):
    proj   = enc[b] @ W_e + dec[b] @ W_d        # [S, A]
    energy = tanh(proj)                          # [S, A]
    scores = energy @ v                          # [S]        (output 1)
    probs  = softmax(scores)
    ctx    = probs @ enc[b]                      # [D]        (output 2)

Device dataflow (per core, 4 batches of S=4096, D=A=512):
  - proj computed transposed (projT[a, s]) on PE: lhsT = W_e chunk [d,a]
    (stationary), rhs = encT chunk [d, s] (bf16, pre-transposed on host).
    encT is the ONLY large input (17 MB/core HBM traffic).
  - dec[b] @ W_d is 0.025% of the FLOPs -> computed on host, folded into the
    tanh as the ACT engine's per-partition bias (projT has a on partitions).
  - scores = v . energyT : matvecs on PE with the stationary v broadcast to
    M in {32,64} columns so each s-tile's scores land replicated at legal
    partition offsets {0,32,64} across 3 PSUM banks (fully initialized).
  - exp on ACT with fused accum_out row sums (softmax Z partials, host-summed).
  - exp rows are replicated across partitions via K=1 PE matmuls
    (ones-row.T @ exps-row), then context = sum_s encT[d,s]*exps[s] runs as
    elementwise mult + pairwise-tree adds + one reduce, split across DVE
    (d-chunks 0-1) and GPSIMD (d-chunks 2-3).
  - Host epilogue: ctx = ctxT / Z (unshard step).
"""

import numpy as np
import ml_dtypes

import concourse.bass as bass
import concourse.bacc as bacc
import concourse.tile as tile
import concourse.mybir as mybir

BF16 = mybir.dt.bfloat16
F32 = mybir.dt.float32
AF = mybir.ActivationFunctionType

N_CORES = 8
B, S, D, A = 32, 4096, 512, 512
BPC = B // N_CORES          # batches per core = 4
P = 128
NDC = D // P                # 4 contraction chunks
NAC = A // P                # 4 a chunks
NST = S // 512              # 8 s tiles of 512

# s-tile -> (psum bank, partition row, replication M)
ST_MAP = [(0, 0, 32), (0, 32, 32), (0, 64, 64),
          (1, 0, 32), (1, 32, 32), (1, 64, 64),
          (2, 0, 64), (2, 64, 64)]


def build_nc():
    nc = bacc.Bacc("TRN2", target_bir_lowering=False, debug=False,
                   num_devices=N_CORES)

    # inputs (per core)
    encT_d = nc.dram_tensor("encT", [BPC, D, S], BF16, kind="ExternalInput")
    we_d = nc.dram_tensor("we", [D, A], BF16, kind="ExternalInput")
    v_d = nc.dram_tensor("vv", [P, NAC], BF16, kind="ExternalInput")
    bias_d = nc.dram_tensor("biasT", [P, NAC * BPC], F32, kind="ExternalInput")
    ones_d = nc.dram_tensor("ones1", [P, P], BF16, kind="ExternalInput")
    id_d = nc.dram_tensor("ident", [P, P], BF16, kind="ExternalInput")
    encN3_d = nc.dram_tensor("encN3", [S, D], BF16, kind="ExternalInput")

    # outputs (per core)
    scores_o = nc.dram_tensor("scores_o", [BPC, S], F32, kind="ExternalOutput")
    ctxT_o = nc.dram_tensor("ctxT_o", [BPC - 1, P, NDC], F32,
                            kind="ExternalOutput")
    ctx3_o = nc.dram_tensor("ctx3_o", [4, D], F32, kind="ExternalOutput")
    zs_o = nc.dram_tensor("zs_o", [BPC, P, 3], F32, kind="ExternalOutput")

    with tile.TileContext(nc) as tc:
        with (
            tc.tile_pool(name="const", bufs=1) as cpool,
            tc.tile_pool(name="encT", bufs=2 * NDC) as encT_pool,
            tc.tile_pool(name="energy", bufs=8) as en_pool,
            tc.tile_pool(name="small", bufs=2) as sm_pool,
            tc.tile_pool(name="scr", bufs=12) as scr_pool,
            tc.tile_pool(name="prep", bufs=4) as prep_pool,
            tc.tile_pool(name="pproj", bufs=2, space="PSUM") as pp_pool,
            tc.tile_pool(name="pscore", bufs=3, space="PSUM") as ps_pool,
            tc.tile_pool(name="prep_ps", bufs=1, space="PSUM") as pr_pool,
        ):
            # constants
            we_sb = cpool.tile([P, NDC * A], BF16, tag="we")
            nc.sync.dma_start(
                out=we_sb[:].rearrange("p (dc a) -> p dc a", dc=NDC),
                in_=we_d[:].rearrange("(dc p) a -> p dc a", p=P),
            )
            v_sb = cpool.tile([P, NAC], BF16, tag="v")
            nc.sync.dma_start(out=v_sb[:], in_=v_d[:])
            bias_sb = cpool.tile([P, NAC * BPC], F32, tag="bias")
            nc.sync.dma_start(out=bias_sb[:], in_=bias_d[:])
            ones_sb = cpool.tile([P, P], BF16, tag="ones1")
            nc.sync.dma_start(out=ones_sb[:], in_=ones_d[:])
            id_sb = cpool.tile([P, P], BF16, tag="ident")
            nc.sync.dma_start(out=id_sb[:], in_=id_d[:])

            for b in range(BPC):
                # ---- load this batch's encoder states (fine-grained so the
                # first proj matmuls can start almost immediately) ----
                encT_sb = [encT_pool.tile([P, S], BF16, tag="encT",
                                          name=f"encT_{b}_{dc}")
                           for dc in range(NDC)]
                nseg = 8 if b == 0 else 2
                seg = S // nseg
                for h in range(nseg):
                    for dc in range(NDC):
                        nc.sync.dma_start(
                            out=encT_sb[dc][:, h * seg:(h + 1) * seg],
                            in_=encT_d[b, dc * P:(dc + 1) * P,
                                       h * seg:(h + 1) * seg])
                if b == BPC - 1:
                    # natural-layout copy of the last batch for its PE context
                    encN3_sb = []
                    for g2 in range(NST):
                        t = encT_pool.tile([P, 4 * 512], BF16, tag="encN3",
                                           name=f"encN3_{g2}")
                        nc.sync.dma_start(
                            out=t[:].rearrange("p (j d) -> p j d", j=4),
                            in_=encN3_d[g2 * 512:(g2 + 1) * 512, :].rearrange(
                                "(j p) d -> p j d", p=P),
                        )
                        encN3_sb.append(t)

                # ---- projT -> tanh -> scores ----
                sc_ps = [ps_pool.tile([P, 512], F32, tag="scores",
                                      name=f"sc_ps_{b}_{g}")
                         for g in range(3)]
                for t2 in range(NST // 2):
                    ets = []
                    for ac in range(NAC):
                        pp = pp_pool.tile([P, 1024], F32, tag="proj",
                                          name=f"pp_{b}_{t2}_{ac}")
                        # dc outer: consecutive matmul pairs share the same
                        # stationary W chunk (halves LDWEIGHTS traffic)
                        for dc in range(NDC):
                            for half in range(2):
                                st = 2 * t2 + half
                                nc.tensor.matmul(
                                    pp[:, half * 512:(half + 1) * 512],
                                    lhsT=we_sb[:, dc * A + ac * P:
                                               dc * A + (ac + 1) * P],
                                    rhs=encT_sb[dc][:, st * 512:(st + 1) * 512],
                                    start=(dc == 0), stop=(dc == NDC - 1),
                                )
                        et = en_pool.tile([P, 1024], BF16, tag="energy",
                                          name=f"et_{b}_{t2}_{ac}")
                        nc.scalar.activation(
                            et[:], pp[:], AF.Tanh,
                            bias=bias_sb[:, ac * BPC + b:ac * BPC + b + 1],
                        )
                        ets.append(et)
                    # one contiguous accumulation group per psum bank
                    for half in range(2):
                        st = 2 * t2 + half
                        g, row, m = ST_MAP[st]
                        for ac in range(NAC):
                            nc.tensor.matmul(
                                sc_ps[g][row:row + m, :],
                                lhsT=v_sb[:, ac:ac + 1].to_broadcast((P, m)),
                                rhs=ets[ac][:, half * 512:(half + 1) * 512],
                                start=(ac == 0), stop=(ac == NAC - 1),
                                tile_position=(0, row),
                            )

                # ---- scores out + exp + softmax partials ----
                exps_sb = sm_pool.tile([P, 3 * 512], BF16, tag="exps")
                zacc_sb = sm_pool.tile([P, 3], F32, tag="zacc")
                for g in range(3):
                    sts = [st for st in range(NST) if ST_MAP[st][0] == g]
                    r = 32 if len(sts) == 3 else 64
                    sc_sb = sm_pool.tile([P, 512], F32, tag="scores_sb",
                                         name=f"sc_sb_{b}_{g}")
                    nc.scalar.copy(sc_sb[:], sc_ps[g][:])
                    nc.sync.dma_start(
                        out=scores_o[b, sts[0] * 512:(sts[-1] + 1) * 512]
                        .rearrange("(j f) -> j f", f=512),
                        in_=sc_sb[:].rearrange("(j r) f -> j (r f)",
                                               r=r)[0:len(sts), 0:512],
                    )
                    nc.scalar.activation(
                        exps_sb[:, g * 512:(g + 1) * 512],
                        sc_ps[g][:],
                        AF.Exp,
                        accum_out=zacc_sb[:, g:g + 1],
                    )
                nc.sync.dma_start(out=zs_o[b], in_=zacc_sb[:])

                if b == BPC - 1:
                    # Last batch: its epilogue is the exposed kernel tail, so
                    # run the context matvec on the (now idle) PE instead.
                    # probsT3[:, st*4+jj] = exps for s-chunk st*4+jj, gathered
                    # by strided SBUF->SBUF DMAs from the exp rows.
                    probsT3 = sm_pool.tile([P, 32], BF16, tag="probsT3")
                    for g in range(3):
                        sts = [st for st in range(NST) if ST_MAP[st][0] == g]
                        rstride = 32 if len(sts) == 3 else 64
                        for jj in range(4):
                            pt_ps = pr_pool.tile([P, P], BF16, tag="prep_ps",
                                                 name=f"pt3_{g}_{jj}")
                            nc.tensor.transpose(
                                pt_ps[:],
                                exps_sb[:, g * 512 + jj * P:
                                        g * 512 + (jj + 1) * P],
                                id_sb[:],
                            )
                            # cols {row(st)} of the transpose are the probs
                            # for s-chunks st*4 + jj
                            nc.vector.tensor_copy(
                                probsT3[:].rearrange("p (cc jj) -> p cc jj",
                                                     jj=4)
                                [:, sts[0]:sts[0] + len(sts), jj:jj + 1],
                                pt_ps[:].rearrange("p (q r) -> p q r",
                                                   r=rstride)
                                [:, 0:len(sts), 0:1],
                            )
                    ctx_ps = pr_pool.tile([P, 512], F32, tag="prep_ps",
                                          name="ctx3_ps")
                    for j in range(4):
                        for i in range(8):
                            c = j * 8 + i
                            nc.tensor.matmul(
                                ctx_ps[32 * j:32 * j + 32, :],
                                lhsT=probsT3[:, c:c + 1].to_broadcast((P, 32)),
                                rhs=encN3_sb[c // 4][:, (c % 4) * 512:
                                                    (c % 4 + 1) * 512],
                                start=(i == 0), stop=(i == 7),
                                tile_position=(0, 32 * j),
                            )
                    ctx3_sb = sm_pool.tile([P, 512], F32, tag="ctx3_sb")
                    nc.scalar.copy(ctx3_sb[:], ctx_ps[:])
                    nc.sync.dma_start(
                        out=ctx3_o[:],
                        in_=ctx3_sb[:].rearrange("(j r) f -> j (r f)",
                                                 r=32)[:, 0:512],
                    )
                    continue

                # ---- replicate exp rows across partitions (K=1 PE matmuls),
                # then context as mult + tree-add + reduce on DVE/GPSIMD ----
                prs = []
                for t2 in range(NST // 2):
                    pr2 = prep_pool.tile([P, 1024], BF16, tag="prep",
                                         name=f"pr2_{b}_{t2}")
                    for half in range(2):
                        st = 2 * t2 + half
                        g, row, _ = ST_MAP[st]
                        pr_ps = pr_pool.tile([P, 512], F32, tag="prep_ps",
                                             name=f"prps_{b}_{st}")
                        nc.tensor.matmul(
                            pr_ps[:],
                            lhsT=ones_sb[row:row + 1, :],
                            rhs=exps_sb[row:row + 1, g * 512:(g + 1) * 512],
                            start=True, stop=True,
                            tile_position=(row, 0),
                        )
                        nc.vector.tensor_copy(
                            pr2[:, half * 512:(half + 1) * 512], pr_ps[:])
                    prs.append(pr2)
                ctxT_sb = sm_pool.tile([P, NDC], F32, tag="ctxT_sb")
                for dc in range(NDC):
                    eng = nc.vector if dc < 3 else nc.gpsimd
                    scrs = []
                    for t2 in range(NST // 2):
                        scr = scr_pool.tile([P, 1024], BF16, tag="scr",
                                            name=f"scr_{b}_{t2}_{dc}")
                        eng.tensor_tensor(
                            out=scr[:],
                            in0=encT_sb[dc][:, t2 * 1024:(t2 + 1) * 1024],
                            in1=prs[t2][:],
                            op=mybir.AluOpType.mult,
                        )
                        scrs.append(scr)
                    eng.tensor_tensor(out=scrs[0][:], in0=scrs[0][:],
                                      in1=scrs[1][:], op=mybir.AluOpType.add)
                    eng.tensor_tensor(out=scrs[2][:], in0=scrs[2][:],
                                      in1=scrs[3][:], op=mybir.AluOpType.add)
                    eng.tensor_tensor(out=scrs[0][:], in0=scrs[0][:],
                                      in1=scrs[2][:], op=mybir.AluOpType.add)
                    nc.vector.reduce_sum(
                        out=ctxT_sb[:, dc:dc + 1], in_=scrs[0][:],
                        axis=mybir.AxisListType.X,
                    )
                nc.sync.dma_start(out=ctxT_o[b], in_=ctxT_sb[:])

    nc.compile()
    return nc


_NC_CACHE = None


def _get_nc():
    global _NC_CACHE
    if _NC_CACHE is None:
        _NC_CACHE = build_nc()
    return _NC_CACHE


def make_in_maps(encode_state, decode_state, W, v):
    """Host-side shard + layout prep. Returns per-core input dicts."""
    bf16 = ml_dtypes.bfloat16
    W_e, W_d = W[:D], W[D:]
    dec_proj = decode_state.astype(np.float32) @ W_d.astype(np.float32)  # [B, A]
    enc_bf = encode_state.astype(bf16)
    we_bf = np.ascontiguousarray(W_e.astype(bf16))
    v_bf = np.ascontiguousarray(v.astype(bf16).reshape(NAC, P).T)        # [P, NAC]
    ones1 = np.ones((P, P), dtype=bf16)

    in_maps = []
    for c in range(N_CORES):
        sl = slice(c * BPC, (c + 1) * BPC)
        encT = np.ascontiguousarray(enc_bf[sl].transpose(0, 2, 1))       # [BPC,D,S]
        # biasT[p, ac*BPC + b] = dec_proj[core b, ac*128 + p]
        biasT = np.ascontiguousarray(
            dec_proj[sl].reshape(BPC, NAC, P).transpose(2, 1, 0).reshape(P, NAC * BPC)
        ).astype(np.float32)
        in_maps.append({
            "encT": encT, "we": we_bf, "vv": v_bf, "biasT": biasT,
            "ones1": ones1, "ident": np.eye(P, dtype=bf16),
            "encN3": np.ascontiguousarray(enc_bf[c * BPC + BPC - 1]),
        })
    return in_maps


def assemble(results):
    """Gather per-core outputs into full (scores, context_vector)."""
    scores = np.concatenate([r["scores_o"] for r in results], axis=0)
    ctx = []
    for r in results:
        zp = r["zs_o"]                                                   # [BPC,P,3]
        z = np.zeros(BPC, dtype=np.float64)
        for st in range(NST):
            g, row, _ = ST_MAP[st]
            z += zp[:, row, g]
        c = np.empty((BPC, D), dtype=np.float64)
        c[:BPC - 1] = r["ctxT_o"].transpose(0, 2, 1).reshape(BPC - 1, D)
        c[BPC - 1] = r["ctx3_o"].sum(axis=0)
        c /= z[:, None]
        ctx.append(c)
    context = np.concatenate(ctx, axis=0).astype(np.float32)
    return scores.astype(np.float32), context


def kernel(encode_state, decode_state, W, v):
    from concourse.bass_utils import run_bass_kernel_spmd
    nc = _get_nc()
    in_maps = make_in_maps(encode_state, decode_state, W, v)
    res = run_bass_kernel_spmd(nc, in_maps, list(range(N_CORES)))
    return assemble(res.results)
